# revision 11
# baseline (speedup 1.0000x reference)
"""Bahdanau attention kernel for Trainium2 (Bass/Tile), 8-core data-parallel.

Problem shapes: B=32, Tx=1024, enc_hid=dec_hid=attn=1024, fp32.

Math (per example b):
  dec_proj = W_dec @ dec_hidden[b]                 [attn]
  energy^T[a, t] = tanh(sum_e W_enc[a,e] enc[b,t,e] + dec_proj[a] + W_b[a])
  scores[t] = sum_a v[a] energy^T[a, t]
  alpha = softmax(mask(scores))
  context[e] = sum_t alpha[t] enc[b,t,e]

Sharding: batch B split 4 examples per core across 8 cores; weights replicated.
All big matmuls run as float32r (fp32 truncated to fp22 in the PE) which at
moving-dim >= 256 streams at full PE rate.

Layouts per core (host-side preprocessing in kernel()):
  encT  [4, E, Tx]   enc transposed  -> e on partitions (enc_proj rhs)
  enc   [4, Tx, E]   natural         -> t on partitions (context rhs)
  w_encT [E, A], w_decT [D, A]       transposed nn.Linear weights
  dec_hT [D, 4], v_col [A, 1], wb8 [128, 8], maskf [4, Tx]
"""

from contextlib import ExitStack

import numpy as np

import concourse.bass as bass
import concourse.tile as tile
from concourse import bacc, mybir

F32 = mybir.dt.float32
F32R = mybir.dt.float32r
AF = mybir.ActivationFunctionType

P = 128
N_CORES = 8
B_LOC = 4            # examples per core
TX = 1024
E = 1024             # enc_hid
A = 1024             # attn
D = 1024             # dec_hid
EO = E // P          # e-chunks
AO = A // P          # a-chunks
TO = TX // P         # t-chunks
DO = D // P          # d-chunks
NT = TX // 512       # t-tiles for energy free dim
ET = E // 512        # e-tiles for context free dim


def _r(ap):
    """View an fp32 AP as float32r for full-rate PE streaming."""
    return ap.bitcast(F32R)


def build_nc():
    nc = bacc.Bacc(
        "TRN2", target_bir_lowering=False, debug=False, num_devices=N_CORES
    )
    encT = nc.dram_tensor("encT", [B_LOC, E, TX], F32, kind="ExternalInput").ap()
    enc = nc.dram_tensor("enc", [B_LOC, TX, E], F32, kind="ExternalInput").ap()
    w_encT = nc.dram_tensor("w_encT", [E, A], F32, kind="ExternalInput").ap()
    w_decT = nc.dram_tensor("w_decT", [D, A], F32, kind="ExternalInput").ap()
    dec_hT = nc.dram_tensor("dec_hT", [D, B_LOC], F32, kind="ExternalInput").ap()
    v_col = nc.dram_tensor("v_col", [A, 1], F32, kind="ExternalInput").ap()
    wb8 = nc.dram_tensor("wb8", [P, AO], F32, kind="ExternalInput").ap()
    maskf = nc.dram_tensor("maskf", [B_LOC, TX], F32, kind="ExternalInput").ap()
    ctx_out = nc.dram_tensor("context", [B_LOC, E], F32, kind="ExternalOutput").ap()
    alpha_out = nc.dram_tensor("alpha", [B_LOC, TX], F32, kind="ExternalOutput").ap()

    with tile.TileContext(nc) as tc, ExitStack() as ctx:
        const = ctx.enter_context(tc.tile_pool(name="const", bufs=1))
        big = ctx.enter_context(tc.tile_pool(name="big", bufs=3))
        en_pool = ctx.enter_context(tc.tile_pool(name="energy", bufs=6))
        small = ctx.enter_context(tc.tile_pool(name="small", bufs=2))
        rowp = ctx.enter_context(tc.tile_pool(name="rows", bufs=2))
        ep_psum = ctx.enter_context(tc.tile_pool(name="ep_ps", bufs=4, space="PSUM"))
        vec_psum = ctx.enter_context(tc.tile_pool(name="vec_ps", bufs=4, space="PSUM"))
        dram = ctx.enter_context(tc.tile_pool(name="dram", bufs=2, space="DRAM"))

        # ---- resident constants. Each DMA-capable engine (sync, scalar
        # HWDGE; gpsimd SWDGE) owns ONE queue at ~175GB/s, and queues are
        # FIFO — so chunks are emitted in NEED order, alternating between the
        # two HW queues. w_decT is only needed once the first energy PSUM
        # groups complete (~24us), and its first ACT consumers need only the
        # a<512 half, so its quarter-chunks trail the w_encT/encT0 pairs.
        w_encT_sb = const.tile([P, EO, A], F32R)
        encT_sb0 = big.tile([P, EO, TX], F32R, tag="big", name="encT_sb0")
        w_decT_sb = big.tile([P, DO, A], F32R, tag="big", name="w_decT_sb")

        def pair_dma(eo):
            nc.sync.dma_start(
                w_encT_sb[:, eo], w_encT[eo * P : (eo + 1) * P, :].bitcast(F32R)
            )
            nc.scalar.dma_start(
                encT_sb0[:, eo], encT[0, eo * P : (eo + 1) * P, :].bitcast(F32R)
            )

        def wdec_dma(do, at, eng):
            cs = slice(at * 512, (at + 1) * 512)
            eng.dma_start(
                w_decT_sb[:, do, cs],
                w_decT[do * P : (do + 1) * P, cs].bitcast(F32R),
            )

        for eo in range(4):
            pair_dma(eo)
        for eo in range(4, EO):
            pair_dma(eo)
            wdec_dma(2 * (eo - 4), 0, nc.sync)
            wdec_dma(2 * (eo - 4) + 1, 0, nc.scalar)
        for do in range(DO):
            wdec_dma(do, 1, nc.sync if do % 2 == 0 else nc.scalar)
        dec_hT_sb = const.tile([P, DO, B_LOC], F32R)
        nc.sync.dma_start(
            dec_hT_sb[:], dec_hT.rearrange("(do p) b -> p do b", p=P).bitcast(F32R)
        )
        v_sb = const.tile([P, AO, 1], F32R)
        nc.sync.dma_start(
            v_sb[:], v_col.rearrange("(ao p) one -> p ao one", p=P).bitcast(F32R)
        )
        wb_sb = const.tile([P, AO], F32)
        nc.sync.dma_start(wb_sb[:], wb8[:])
        bias_sb = const.tile([P, AO, B_LOC], F32)

        # ---- dec_proj + bias: bias[a, b] = W_dec @ dec_h[b] + W_b ---------
        # b-stationary f32r form: lhsT = dec_h^T chunk [128d, 4b] (4-col weight
        # load), rhs = w_decT chunk [128d, 512a] -> psum [4b, 512a], then a
        # DRAM bounce transposes [4, A] into the [a-partition, b] bias layout.
        dp_row = rowp.tile([B_LOC, A], F32, tag="dprow")
        for at in range(A // 512):
            dp_ps = ep_psum.tile([P, 512], F32, tag="ep", name=f"dp_ps{at}")
            for do in range(DO):
                nc.tensor.matmul(
                    dp_ps[:B_LOC, :],
                    lhsT=dec_hT_sb[:, do],
                    rhs=w_decT_sb[:, do, at * 512 : (at + 1) * 512],
                    start=(do == 0),
                    stop=(do == DO - 1),
                )
            nc.vector.tensor_copy(dp_row[:, at * 512 : (at + 1) * 512], dp_ps[:B_LOC, :])
        dp_scr = dram.tile([B_LOC, A], F32, tag="dpscr")
        nc.sync.dma_start(dp_scr[:], dp_row[:])
        dpT = small.tile([P, AO, B_LOC], F32, tag="dpT")
        dp_scr_t = dp_scr.rearrange("b (ao p) -> p ao b", p=P)
        for ao in range(AO):
            nc.sync.dma_start(dpT[:, ao], dp_scr_t[:, ao])
        for ao in range(AO):
            nc.vector.tensor_scalar_add(
                bias_sb[:, ao], dpT[:, ao], wb_sb[:, ao : ao + 1]
            )

        # ---- per-example pipeline -----------------------------------------
        # tile_wait_until gates keep prefetch DMAs from stealing bandwidth
        # from the start-critical stream; values are model-time estimates of
        # when each tile is actually needed (conservatively early).
        for b in range(B_LOC):
            if b == 0:
                encT_sb = encT_sb0
            else:
                encT_sb = big.tile([P, EO, TX], F32R, tag="big", name=f"encT_sb{b}")
                big_dma = [nc.sync, nc.scalar, nc.gpsimd]
                for eo in range(EO):
                    big_dma[eo % 3].dma_start(
                        encT_sb[:, eo],
                        encT[b, eo * P : (eo + 1) * P, :].bitcast(F32R),
                    )
            mask_row = small.tile([1, TX], F32, tag="mrow", name=f"mask{b}")
            nc.sync.dma_start(mask_row[:], maskf[b : b + 1, :])

            # energy^T tiles + score accumulation
            sc_ps = [
                vec_psum.tile([1, 512], F32, tag="vec", name=f"sc{b}_{nt}")
                for nt in range(NT)
            ]
            for ao in range(AO):
                for nt in range(NT):
                    ep_ps = ep_psum.tile([P, 512], F32, tag="ep", name=f"ep{b}_{ao}_{nt}")
                    for eo in range(EO):
                        nc.tensor.matmul(
                            ep_ps[:],
                            lhsT=w_encT_sb[:, eo, ao * P : (ao + 1) * P],
                            rhs=encT_sb[:, eo, nt * 512 : (nt + 1) * 512],
                            start=(eo == 0),
                            stop=(eo == EO - 1),
                        )
                    energy = en_pool.tile(
                        [P, 512], F32R, tag="energy", name=f"en{b}_{ao}_{nt}"
                    )
                    nc.scalar.activation(
                        energy[:], ep_ps[:], AF.Tanh, bias=bias_sb[:, ao, b : b + 1]
                    )
                    nc.tensor.matmul(
                        sc_ps[nt][:],
                        lhsT=v_sb[:, ao],
                        rhs=energy[:],
                        start=(ao == 0),
                        stop=(ao == AO - 1),
                    )

            # softmax with masking (all on partition 0). Scores are bounded
            # (|s| <= sum|v| ~ 26 since |tanh| <= 1), so exp needs no max
            # shift -- softmax is shift-invariant and exp(26) is safe in f32.
            # The exp -> mask -> DRAM-bounce transpose chain is pipelined per
            # 512-half so context matmuls on the first t-chunks can overlap
            # the second half. (The bounce exists because sbuf->sbuf
            # partition-crossing reshapes don't balance as DMA APs.)
            exp_row = rowp.tile([1, TX], F32, tag="erow", name=f"exp{b}")
            exp_scr = dram.tile([TX], F32, tag="escr", name=f"escr{b}")
            expT = small.tile([P, TO], F32R, tag="expT", name=f"expT{b}")
            exp_scr_t = exp_scr.rearrange("(to p) -> p to", p=P).bitcast(F32R)
            HTO = TO // NT
            for nt in range(NT):
                hs = slice(nt * 512, (nt + 1) * 512)
                nc.scalar.activation(exp_row[:, hs], sc_ps[nt][:], AF.Exp)
                nc.vector.tensor_mul(
                    out=exp_row[:, hs], in0=exp_row[:, hs], in1=mask_row[:, hs]
                )
                eng = nc.sync if nt == 0 else nc.scalar
                eng.dma_start(exp_scr[None, hs], exp_row[:, hs])
                eng.dma_start(
                    expT[:, nt * HTO : (nt + 1) * HTO],
                    exp_scr_t[:, nt * HTO : (nt + 1) * HTO],
                )

            ssum = small.tile([1, 1], F32, tag="ssum", name=f"ssum{b}")
            nc.vector.reduce_sum(ssum[:], exp_row[:], axis=mybir.AxisListType.X)
            rsum = small.tile([1, 1], F32, tag="rsum", name=f"rsum{b}")
            nc.vector.reciprocal(rsum[:], ssum[:])
            alpha_row = rowp.tile([1, TX], F32, tag="arow", name=f"alpha{b}")
            nc.vector.tensor_scalar_mul(alpha_row[:], exp_row[:], rsum[:])
            nc.sync.dma_start(alpha_out[b : b + 1, :], alpha_row[:])

            # context[e] = sum_t alpha[t] enc[t, e] (normalize on evacuation)
            enc_nat = big.tile([P, TO, E], F32R, tag="big", name=f"encN_sb{b}")
            big_dma = [nc.sync, nc.scalar, nc.gpsimd]
            for to in range(TO):
                big_dma[to % 3].dma_start(
                    enc_nat[:, to], enc[b, to * P : (to + 1) * P, :].bitcast(F32R)
                )
            ctx_row = rowp.tile([1, E], F32, tag="crow", name=f"ctx{b}")
            for et in range(ET):
                cx_ps = vec_psum.tile([1, 512], F32, tag="vec", name=f"cx{b}_{et}")
                for to in range(TO):
                    nc.tensor.matmul(
                        cx_ps[:],
                        lhsT=expT[:, to : to + 1],
                        rhs=enc_nat[:, to, et * 512 : (et + 1) * 512],
                        start=(to == 0),
                        stop=(to == TO - 1),
                    )
                nc.vector.tensor_scalar_mul(
                    ctx_row[:, et * 512 : (et + 1) * 512], cx_ps[:], rsum[:]
                )
            nc.sync.dma_start(ctx_out[b : b + 1, :], ctx_row[:])

    nc.compile()
    return nc


_NC = None


def _get_nc():
    global _NC
    if _NC is None:
        _NC = build_nc()
    return _NC


def make_in_maps(dec_hidden, enc_outputs, mask, W_w, W_b, v_w):
    dec_hidden = np.asarray(dec_hidden, np.float32)
    enc_outputs = np.asarray(enc_outputs, np.float32)
    W_w = np.asarray(W_w, np.float32)
    W_b = np.asarray(W_b, np.float32)
    v_w = np.asarray(v_w, np.float32)
    maskf = np.asarray(mask).astype(np.float32)

    enc = np.ascontiguousarray(enc_outputs)
    encT = np.ascontiguousarray(enc_outputs.transpose(0, 2, 1))
    w_encT = np.ascontiguousarray(W_w[:, D:].T)
    w_decT = np.ascontiguousarray(W_w[:, :D].T)
    wb8 = np.ascontiguousarray(W_b.reshape(AO, P).T)
    v_col = np.ascontiguousarray(v_w.reshape(A, 1))

    in_maps = []
    for c in range(N_CORES):
        sl = slice(B_LOC * c, B_LOC * (c + 1))
        in_maps.append(
            {
                "encT": encT[sl],
                "enc": enc[sl],
                "w_encT": w_encT,
                "w_decT": w_decT,
                "dec_hT": np.ascontiguousarray(dec_hidden[sl].T),
                "v_col": v_col,
                "wb8": wb8,
                "maskf": np.ascontiguousarray(maskf[sl]),
            }
        )
    return in_maps


def kernel(dec_hidden, enc_outputs, mask, W_w, W_b, v_w):
    from concourse.bass_utils import run_bass_kernel_spmd

    assert enc_outputs.shape == (N_CORES * B_LOC, TX, E), enc_outputs.shape
    nc = _get_nc()
    in_maps = make_in_maps(dec_hidden, enc_outputs, mask, W_w, W_b, v_w)
    res = run_bass_kernel_spmd(nc, in_maps, list(range(N_CORES))).results
    context = np.concatenate([res[c]["context"] for c in range(N_CORES)], axis=0)
    alpha = np.concatenate([res[c]["alpha"] for c in range(N_CORES)], axis=0)
    return context, alpha


# revision 13
# speedup vs baseline: 1.0738x; 1.0738x over previous
"""Bahdanau attention kernel for Trainium2 (Bass/Tile), 8-core data-parallel.

Problem shapes: B=32, Tx=1024, enc_hid=dec_hid=attn=1024, fp32.

Math (per example b):
  dec_proj = W_dec @ dec_hidden[b]                 [attn]
  energy^T[a, t] = tanh(sum_e W_enc[a,e] enc[b,t,e] + dec_proj[a] + W_b[a])
  scores[t] = sum_a v[a] energy^T[a, t]
  alpha = softmax(mask(scores))
  context[e] = sum_t alpha[t] enc[b,t,e]

Sharding: batch B split 4 examples per core across 8 cores; weights replicated.
All big matmuls run as float32r (fp32 truncated to fp22 in the PE) which at
moving-dim >= 256 streams at full PE rate.

Layouts per core (host-side preprocessing in kernel()):
  encT  [4, E, Tx]   enc transposed  -> e on partitions (enc_proj rhs)
  enc   [4, Tx, E]   natural         -> t on partitions (context rhs)
  w_encT [E, A], w_decT [D, A]       transposed nn.Linear weights
  dec_hT [D, 4], v_col [A, 1], wb8 [128, 8], maskf [4, Tx]
"""

from contextlib import ExitStack

import numpy as np

import concourse.bass as bass
import concourse.tile as tile
from concourse import bacc, mybir

F32 = mybir.dt.float32
F32R = mybir.dt.float32r
AF = mybir.ActivationFunctionType

P = 128
N_CORES = 8
B_LOC = 4            # examples per core
TX = 1024
E = 1024             # enc_hid
A = 1024             # attn
D = 1024             # dec_hid
EO = E // P          # e-chunks
AO = A // P          # a-chunks
TO = TX // P         # t-chunks
DO = D // P          # d-chunks
NT = TX // 512       # t-tiles for energy free dim
ET = E // 512        # e-tiles for context free dim


def _r(ap):
    """View an fp32 AP as float32r for full-rate PE streaming."""
    return ap.bitcast(F32R)


def build_nc():
    nc = bacc.Bacc(
        "TRN2", target_bir_lowering=False, debug=False, num_devices=N_CORES
    )
    encT = nc.dram_tensor("encT", [B_LOC, E, TX], F32, kind="ExternalInput").ap()
    enc = nc.dram_tensor("enc", [B_LOC, TX, E], F32, kind="ExternalInput").ap()
    w_encT = nc.dram_tensor("w_encT", [E, A], F32, kind="ExternalInput").ap()
    w_decT = nc.dram_tensor("w_decT", [D, A], F32, kind="ExternalInput").ap()
    dec_hT = nc.dram_tensor("dec_hT", [D, B_LOC], F32, kind="ExternalInput").ap()
    v_col = nc.dram_tensor("v_col", [A, 1], F32, kind="ExternalInput").ap()
    wb8 = nc.dram_tensor("wb8", [P, AO], F32, kind="ExternalInput").ap()
    maskf = nc.dram_tensor("maskf", [B_LOC, TX], F32, kind="ExternalInput").ap()
    ctx_out = nc.dram_tensor("context", [B_LOC, E], F32, kind="ExternalOutput").ap()
    alpha_out = nc.dram_tensor("alpha", [B_LOC, TX], F32, kind="ExternalOutput").ap()

    with tile.TileContext(nc) as tc, ExitStack() as ctx:
        const = ctx.enter_context(tc.tile_pool(name="const", bufs=1))
        big = ctx.enter_context(tc.tile_pool(name="big", bufs=4))
        en_pool = ctx.enter_context(tc.tile_pool(name="energy", bufs=6))
        small = ctx.enter_context(tc.tile_pool(name="small", bufs=2))
        rowp = ctx.enter_context(tc.tile_pool(name="rows", bufs=2))
        ep_psum = ctx.enter_context(tc.tile_pool(name="ep_ps", bufs=4, space="PSUM"))
        vec_psum = ctx.enter_context(tc.tile_pool(name="vec_ps", bufs=4, space="PSUM"))
        dram = ctx.enter_context(tc.tile_pool(name="dram", bufs=2, space="DRAM"))

        # ---- resident constants. Queue model: sync/scalar HWDGE ~200GB/s
        # each, gpsimd SWDGE ~130GB/s, all FIFO. The dec_proj matmuls head
        # the in-order PE stream, so w_decT (split in column halves) must
        # land FIRST on each HW queue; the w_encT/encT0 pairs follow and
        # pace the b=0 energy groups; gpsimd takes the small/latency traffic
        # plus encN0.
        w_encT_sb = const.tile([P, EO, A], F32R)
        encT_sb0 = big.tile([P, EO, TX], F32R, tag="big", name="encT_sb0")
        w_decT_sb = big.tile([P, DO, A], F32R, tag="big", name="w_decT_sb")
        for do in range(DO):
            nc.sync.dma_start(
                w_decT_sb[:, do, 0:512],
                w_decT[do * P : (do + 1) * P, 0:512].bitcast(F32R),
            )
            nc.scalar.dma_start(
                w_decT_sb[:, do, 512:1024],
                w_decT[do * P : (do + 1) * P, 512:1024].bitcast(F32R),
            )
        for eo in range(EO):
            nc.sync.dma_start(
                w_encT_sb[:, eo], w_encT[eo * P : (eo + 1) * P, :].bitcast(F32R)
            )
            nc.scalar.dma_start(
                encT_sb0[:, eo], encT[0, eo * P : (eo + 1) * P, :].bitcast(F32R)
            )
        dec_hT_sb = const.tile([P, DO, B_LOC], F32R)
        nc.gpsimd.dma_start(
            dec_hT_sb[:], dec_hT.rearrange("(do p) b -> p do b", p=P).bitcast(F32R)
        )
        v_sb = const.tile([P, AO, 1], F32R)
        nc.gpsimd.dma_start(
            v_sb[:], v_col.rearrange("(ao p) one -> p ao one", p=P).bitcast(F32R)
        )
        wb_sb = const.tile([P, AO], F32)
        nc.gpsimd.dma_start(wb_sb[:], wb8[:])
        bias_sb = const.tile([P, AO, B_LOC], F32)

        # ---- dec_proj + bias: bias[a, b] = W_dec @ dec_h[b] + W_b ---------
        # b-stationary f32r form: lhsT = dec_h^T chunk [128d, 4b] (4-col weight
        # load), rhs = w_decT chunk [128d, 512a] -> psum [4b, 512a], then a
        # DRAM bounce transposes [4, A] into the [a-partition, b] bias layout.
        dp_row = rowp.tile([B_LOC, A], F32, tag="dprow")
        for at in range(A // 512):
            dp_ps = ep_psum.tile([P, 512], F32, tag="ep", name=f"dp_ps{at}")
            for do in range(DO):
                nc.tensor.matmul(
                    dp_ps[:B_LOC, :],
                    lhsT=dec_hT_sb[:, do],
                    rhs=w_decT_sb[:, do, at * 512 : (at + 1) * 512],
                    start=(do == 0),
                    stop=(do == DO - 1),
                )
            nc.vector.tensor_copy(dp_row[:, at * 512 : (at + 1) * 512], dp_ps[:B_LOC, :])
        dp_scr = dram.tile([B_LOC, A], F32, tag="dpscr")
        nc.gpsimd.dma_start(dp_scr[:], dp_row[:])
        dpT = small.tile([P, AO, B_LOC], F32, tag="dpT")
        dp_scr_t = dp_scr.rearrange("b (ao p) -> p ao b", p=P)
        for ao in range(AO):
            nc.gpsimd.dma_start(dpT[:, ao], dp_scr_t[:, ao])
        for ao in range(AO):
            nc.vector.tensor_scalar_add(
                bias_sb[:, ao], dpT[:, ao], wb_sb[:, ao : ao + 1]
            )

        # ---- per-example pipeline -----------------------------------------
        # tile_wait_until gates keep prefetch DMAs from stealing bandwidth
        # from the start-critical stream; values are model-time estimates of
        # when each tile is actually needed (conservatively early).
        for b in range(B_LOC):
            if b == 0:
                encT_sb = encT_sb0
            else:
                encT_sb = big.tile([P, EO, TX], F32R, tag="big", name=f"encT_sb{b}")
                for eo in range(EO):
                    eng = nc.sync if eo % 2 == 0 else nc.scalar
                    eng.dma_start(
                        encT_sb[:, eo],
                        encT[b, eo * P : (eo + 1) * P, :].bitcast(F32R),
                    )
            mask_row = small.tile([1, TX], F32, tag="mrow", name=f"mask{b}")
            nc.gpsimd.dma_start(mask_row[:], maskf[b : b + 1, :])

            # energy^T tiles + score accumulation
            sc_ps = [
                vec_psum.tile([1, 512], F32, tag="vec", name=f"sc{b}_{nt}")
                for nt in range(NT)
            ]
            for ao in range(AO):
                for nt in range(NT):
                    ep_ps = ep_psum.tile([P, 512], F32, tag="ep", name=f"ep{b}_{ao}_{nt}")
                    for eo in range(EO):
                        nc.tensor.matmul(
                            ep_ps[:],
                            lhsT=w_encT_sb[:, eo, ao * P : (ao + 1) * P],
                            rhs=encT_sb[:, eo, nt * 512 : (nt + 1) * 512],
                            start=(eo == 0),
                            stop=(eo == EO - 1),
                        )
                    energy = en_pool.tile(
                        [P, 512], F32R, tag="energy", name=f"en{b}_{ao}_{nt}"
                    )
                    nc.scalar.activation(
                        energy[:], ep_ps[:], AF.Tanh, bias=bias_sb[:, ao, b : b + 1]
                    )
                    nc.tensor.matmul(
                        sc_ps[nt][:],
                        lhsT=v_sb[:, ao],
                        rhs=energy[:],
                        start=(ao == 0),
                        stop=(ao == AO - 1),
                    )

            # softmax with masking (all on partition 0). Scores are bounded
            # (|s| <= sum|v| ~ 26 since |tanh| <= 1), so exp needs no max
            # shift -- softmax is shift-invariant and exp(26) is safe in f32.
            # The exp -> mask -> DRAM-bounce transpose chain is pipelined per
            # 512-half so context matmuls on the first t-chunks can overlap
            # the second half. (The bounce exists because sbuf->sbuf
            # partition-crossing reshapes don't balance as DMA APs.)
            exp_row = rowp.tile([1, TX], F32, tag="erow", name=f"exp{b}")
            exp_scr = dram.tile([TX], F32, tag="escr", name=f"escr{b}")
            expT = small.tile([P, TO], F32R, tag="expT", name=f"expT{b}")
            exp_scr_t = exp_scr.rearrange("(to p) -> p to", p=P).bitcast(F32R)
            HTO = TO // NT
            for nt in range(NT):
                hs = slice(nt * 512, (nt + 1) * 512)
                nc.scalar.activation(exp_row[:, hs], sc_ps[nt][:], AF.Exp)
                nc.vector.tensor_mul(
                    out=exp_row[:, hs], in0=exp_row[:, hs], in1=mask_row[:, hs]
                )
                eng = nc.sync if nt == 0 else nc.scalar
                eng.dma_start(exp_scr[None, hs], exp_row[:, hs])
                eng.dma_start(
                    expT[:, nt * HTO : (nt + 1) * HTO],
                    exp_scr_t[:, nt * HTO : (nt + 1) * HTO],
                )

            ssum = small.tile([1, 1], F32, tag="ssum", name=f"ssum{b}")
            nc.vector.reduce_sum(ssum[:], exp_row[:], axis=mybir.AxisListType.X)
            rsum = small.tile([1, 1], F32, tag="rsum", name=f"rsum{b}")
            nc.vector.reciprocal(rsum[:], ssum[:])
            # normalize in place (the expT bounce DMAs have already read
            # exp_row; Tile orders the WAR dependency)
            nc.vector.tensor_scalar_mul(exp_row[:], exp_row[:], rsum[:])
            nc.sync.dma_start(alpha_out[b : b + 1, :], exp_row[:])

            # context[e] = sum_t alpha[t] enc[t, e] (normalize on evacuation)
            enc_nat = big.tile([P, TO, E], F32R, tag="big", name=f"encN_sb{b}")
            for to in range(TO):
                nc.gpsimd.dma_start(
                    enc_nat[:, to], enc[b, to * P : (to + 1) * P, :].bitcast(F32R)
                )
            ctx_row = rowp.tile([1, E], F32, tag="crow", name=f"ctx{b}")
            for et in range(ET):
                cx_ps = vec_psum.tile([1, 512], F32, tag="vec", name=f"cx{b}_{et}")
                for to in range(TO):
                    nc.tensor.matmul(
                        cx_ps[:],
                        lhsT=expT[:, to : to + 1],
                        rhs=enc_nat[:, to, et * 512 : (et + 1) * 512],
                        start=(to == 0),
                        stop=(to == TO - 1),
                    )
                nc.vector.tensor_scalar_mul(
                    ctx_row[:, et * 512 : (et + 1) * 512], cx_ps[:], rsum[:]
                )
            nc.sync.dma_start(ctx_out[b : b + 1, :], ctx_row[:])

    nc.compile()
    return nc


_NC = None


def _get_nc():
    global _NC
    if _NC is None:
        _NC = build_nc()
    return _NC


def make_in_maps(dec_hidden, enc_outputs, mask, W_w, W_b, v_w):
    dec_hidden = np.asarray(dec_hidden, np.float32)
    enc_outputs = np.asarray(enc_outputs, np.float32)
    W_w = np.asarray(W_w, np.float32)
    W_b = np.asarray(W_b, np.float32)
    v_w = np.asarray(v_w, np.float32)
    maskf = np.asarray(mask).astype(np.float32)

    enc = np.ascontiguousarray(enc_outputs)
    encT = np.ascontiguousarray(enc_outputs.transpose(0, 2, 1))
    w_encT = np.ascontiguousarray(W_w[:, D:].T)
    w_decT = np.ascontiguousarray(W_w[:, :D].T)
    wb8 = np.ascontiguousarray(W_b.reshape(AO, P).T)
    v_col = np.ascontiguousarray(v_w.reshape(A, 1))

    in_maps = []
    for c in range(N_CORES):
        sl = slice(B_LOC * c, B_LOC * (c + 1))
        in_maps.append(
            {
                "encT": encT[sl],
                "enc": enc[sl],
                "w_encT": w_encT,
                "w_decT": w_decT,
                "dec_hT": np.ascontiguousarray(dec_hidden[sl].T),
                "v_col": v_col,
                "wb8": wb8,
                "maskf": np.ascontiguousarray(maskf[sl]),
            }
        )
    return in_maps


def kernel(dec_hidden, enc_outputs, mask, W_w, W_b, v_w):
    from concourse.bass_utils import run_bass_kernel_spmd

    assert enc_outputs.shape == (N_CORES * B_LOC, TX, E), enc_outputs.shape
    nc = _get_nc()
    in_maps = make_in_maps(dec_hidden, enc_outputs, mask, W_w, W_b, v_w)
    res = run_bass_kernel_spmd(nc, in_maps, list(range(N_CORES))).results
    context = np.concatenate([res[c]["context"] for c in range(N_CORES)], axis=0)
    alpha = np.concatenate([res[c]["alpha"] for c in range(N_CORES)], axis=0)
    return context, alpha
